# revision 8
# baseline (speedup 1.0000x reference)
"""Depthwise Conv1d (B=8, L=4096, C=2048, K=4, PAD=3) Bass kernel for 8 trn2 cores.

Sharding: batch-parallel, one batch element per NeuronCore (B == n_cores == 8).

Per-core dataflow (exact fp32):
  - DMA x in [128l, 2, 2048c] row-tiles (fully contiguous 2MiB, HBM line-rate)
  - PE transposes 128x128 blocks into channel-major PSUM tiles
    pin[128c, 3+512 l]; the 3-column halo comes from a tiny PE re-transpose
    of the previous chunk's last 3 rows (zeros via memset at the edges)
  - conv taps are shifted free-dim slices of pin:
      acc  = Copy(pin[:, 0:512]*w0 + bias)      (ACT, per-partition scale+bias)
      acc += pin[:, k:512+k]*wk                 (DVE scalar_tensor_tensor), or
                                                (ACT mult + GPSIMD add) for a
                                                subset of c-blocks to balance
  - PE transposes acc back -> PSUM -> ACT copies -> contiguous DMA out
"""

import sys

for _p in ("/opt/trn_rl_repo",):
    if _p not in sys.path:
        sys.path.insert(0, _p)

import numpy as np

import concourse.bass as bass  # noqa: F401  (registers rust bindings)
import concourse.tile as tile
from concourse import bacc, mybir
from concourse.bass_utils import run_bass_kernel_spmd

F32 = mybir.dt.float32
MULT = mybir.AluOpType.mult
ADD = mybir.AluOpType.add
COPY = mybir.ActivationFunctionType.Copy

B, L, C, K, PAD = 8, 4096, 2048, 4, 3
LOUT = L + 2 * PAD - K + 1  # 4099
NCB = C // 128  # 16 channel blocks
CHUNK = 512  # output rows per main chunk
NCHUNK = L // CHUNK  # 8; last chunk extended to cover the LOUT tail
TAIL = LOUT - L  # 3 extra output rows in the last chunk

# tap-1 routing: c-blocks with index < ACT_TAP1_CBS use ACT-mult + GPSIMD-add;
# the rest use a fused DVE scalar_tensor_tensor. Taps 2 and 3 are always DVE.
ACT_TAP1_CBS = 12


def _build_nc():
    nc = bacc.Bacc("TRN2", target_bir_lowering=False, num_devices=B)

    x_d = nc.dram_tensor("x", [L, C], F32, kind="ExternalInput")
    wt_d = nc.dram_tensor("wt", [128, NCB * K], F32, kind="ExternalInput")
    bt_d = nc.dram_tensor("bt", [128, NCB], F32, kind="ExternalInput")
    id_d = nc.dram_tensor("ident", [128, 128], F32, kind="ExternalInput")
    out_d = nc.dram_tensor("out", [LOUT, C], F32, kind="ExternalOutput")

    with tile.TileContext(nc) as tc:
        with (
            tc.tile_pool(name="const", bufs=1) as cpool,
            tc.tile_pool(name="xin", bufs=4) as xin_pool,
            tc.tile_pool(name="t1", bufs=4) as t1_pool,
            tc.tile_pool(name="acc", bufs=1) as acc_pool,
            tc.tile_pool(name="osb", bufs=2) as out_pool,
            tc.tile_pool(name="pin", bufs=2, space="PSUM") as pin_pool,
            tc.tile_pool(name="pout", bufs=2, space="PSUM") as pout_pool,
        ):
            wt_sb = cpool.tile([128, NCB * K], F32)
            nc.sync.dma_start(out=wt_sb[:], in_=wt_d[:])
            bt_sb = cpool.tile([128, NCB], F32)
            nc.sync.dma_start(out=bt_sb[:], in_=bt_d[:])
            ident = cpool.tile([128, 128], F32)
            nc.sync.dma_start(out=ident[:], in_=id_d[:])

            for ci in range(NCHUNK):
                last = ci == NCHUNK - 1
                r0 = ci * CHUNK
                width = CHUNK + (TAIL if last else 0)  # output rows this chunk
                pinw = width + PAD  # pin columns

                # ---- load 512 input rows as 2x 2MiB contiguous DMAs ----
                xs = []
                for pair in range(2):
                    t = xin_pool.tile([128, 2, C], F32, tag="xin")
                    src = x_d[r0 + pair * 256 : r0 + pair * 256 + 256, :]
                    nc.sync.dma_start(
                        out=t[:], in_=src.rearrange("(r p) c -> p r c", p=128)
                    )
                    xs.append(t)
                # 3 halo rows [r0-3, r0) re-read as a tiny DMA (24KB)
                halo_sb = None
                if ci > 0:
                    halo_sb = xin_pool.tile([PAD, C], F32, tag="halo")
                    nc.sync.dma_start(out=halo_sb[:], in_=x_d[r0 - PAD : r0, :])

                # ---- per c-block: transpose into PSUM, then conv taps ----
                # PE writes must not cross the 2KB PSUM bank boundary, so the
                # data starts at column BASE=125: the 3 halo columns occupy
                # [125,128) and each 128-wide transpose lands bank-aligned.
                BASE = 128 - PAD
                accs = []
                for cb in range(NCB):
                    cs = slice(cb * 128, (cb + 1) * 128)
                    pin = pin_pool.tile([128, BASE + pinw], F32, tag="pin")
                    # halo: x rows [r0-3, r0) transposed into pin[:, BASE:BASE+3]
                    if ci == 0:
                        nc.vector.memset(pin[:, BASE : BASE + PAD], 0.0)
                    else:
                        nc.tensor.transpose(
                            pin[:, BASE : BASE + PAD],
                            halo_sb[0:PAD, cs],
                            ident[0:PAD, 0:PAD],
                        )
                    for lb in range(4):
                        nc.tensor.transpose(
                            pin[:, 128 + lb * 128 : 128 + (lb + 1) * 128],
                            xs[lb // 2][:, lb % 2, cs],
                            ident[:],
                        )
                    if last:
                        # virtual zero-pad rows beyond the end of x
                        nc.vector.memset(pin[:, 128 + CHUNK : BASE + pinw], 0.0)

                    wk = lambda k: wt_sb[:, cb * K + k : cb * K + k + 1]

                    acc = acc_pool.tile([128, width], F32, tag=f"acc{cb}")
                    # tap 0 on ACT: acc = pin[:, 0:w]*w0 + bias
                    # (Identity, not Copy: Copy rejects an AP bias)
                    nc.scalar.activation(
                        out=acc[:],
                        in_=pin[:, BASE : BASE + width],
                        func=mybir.ActivationFunctionType.Identity,
                        scale=wk(0),
                        bias=bt_sb[:, cb : cb + 1],
                    )
                    # taps 2, 3 fused on DVE
                    for k in (2, 3):
                        nc.vector.scalar_tensor_tensor(
                            out=acc[:],
                            in0=pin[:, BASE + k : BASE + k + width],
                            scalar=wk(k),
                            in1=acc[:],
                            op0=MULT,
                            op1=ADD,
                        )
                    # tap 1: ACT-mult + GPSIMD-add for most c-blocks
                    if cb < ACT_TAP1_CBS:
                        t1 = t1_pool.tile([128, width], F32, tag="t1")
                        nc.scalar.activation(
                            out=t1[:],
                            in_=pin[:, BASE + 1 : BASE + 1 + width],
                            func=COPY,
                            scale=wk(1),
                        )
                        nc.gpsimd.tensor_add(out=acc[:], in0=acc[:], in1=t1[:])
                    else:
                        nc.vector.scalar_tensor_tensor(
                            out=acc[:],
                            in0=pin[:, BASE + 1 : BASE + 1 + width],
                            scalar=wk(1),
                            in1=acc[:],
                            op0=MULT,
                            op1=ADD,
                        )
                    accs.append(acc)

                prev_last_x = xs[1]

                # ---- transpose back + store ----
                for pair in range(2):
                    osb = out_pool.tile([128, 2, C], F32, tag="osb")
                    for sub in range(2):
                        lb = pair * 2 + sub
                        for ch in range(2):
                            po = pout_pool.tile([128, 1024], F32, tag="pout")
                            for j in range(8):
                                cb = ch * 8 + j
                                nc.tensor.transpose(
                                    po[:, j * 128 : (j + 1) * 128],
                                    accs[cb][:, lb * 128 : (lb + 1) * 128],
                                    ident[:],
                                )
                            nc.scalar.copy(
                                out=osb[:, sub, ch * 1024 : (ch + 1) * 1024],
                                in_=po[:],
                            )
                    dst = out_d[r0 + pair * 256 : r0 + pair * 256 + 256, :]
                    nc.sync.dma_start(
                        out=dst.rearrange("(r p) c -> p r c", p=128), in_=osb[:]
                    )

                if last:
                    # final TAIL output rows [L, LOUT)
                    osb3 = out_pool.tile([TAIL, C], F32, tag="osb")
                    for ch in range(2):
                        po = pout_pool.tile([TAIL, 1024], F32, tag="pout")
                        for j in range(8):
                            cb = ch * 8 + j
                            nc.tensor.transpose(
                                po[:, j * 128 : (j + 1) * 128],
                                accs[cb][:, CHUNK : CHUNK + TAIL],
                                ident[:],
                            )
                        nc.scalar.copy(
                            out=osb3[:, ch * 1024 : (ch + 1) * 1024], in_=po[:]
                        )
                    nc.sync.dma_start(out=out_d[L:LOUT, :], in_=osb3[:])

    nc.compile()
    return nc


_NC_CACHE = None


def _get_nc():
    global _NC_CACHE
    if _NC_CACHE is None:
        _NC_CACHE = _build_nc()
    return _NC_CACHE


def _const_inputs(weight, bias):
    # wt[p, cb*K + k] = weight[cb*128 + p, k]
    wt = np.ascontiguousarray(
        weight.astype(np.float32).reshape(NCB, 128, K).transpose(1, 0, 2)
    ).reshape(128, NCB * K)
    bt = np.ascontiguousarray(bias.astype(np.float32).reshape(NCB, 128).T)
    ident = np.eye(128, dtype=np.float32)
    return wt, bt, ident


def kernel(x, weight, bias):
    assert x.shape == (B, L, C) and weight.shape == (C, K) and bias.shape == (C,)
    nc = _get_nc()
    wt, bt, ident = _const_inputs(weight, bias)
    in_maps = [
        {
            "x": np.ascontiguousarray(x[b], dtype=np.float32),
            "wt": wt,
            "bt": bt,
            "ident": ident,
        }
        for b in range(B)
    ]
    res = run_bass_kernel_spmd(nc, in_maps, core_ids=list(range(B)))
    return np.stack([res.results[b]["out"] for b in range(B)], axis=0)


if __name__ == "__main__":
    rng = np.random.default_rng(0)
    x = rng.standard_normal((B, L, C), dtype=np.float32)
    w = (rng.standard_normal((C, K)) * 0.1).astype(np.float32)
    bias = (rng.standard_normal((C,)) * 0.1).astype(np.float32)
    out = kernel(x, w, bias)
    print("out", out.shape, out.dtype)


# revision 9
# speedup vs baseline: 2.0172x; 2.0172x over previous
"""Depthwise Conv1d (B=8, L=4096, C=2048, K=4, PAD=3) Bass kernel for 8 trn2 cores.

Sharding: batch-parallel, one batch element per NeuronCore (B == n_cores == 8).

Per-core dataflow (exact fp32):
  - DMA x in [128l, 2, 2048c] row-tiles (fully contiguous 2MiB, HBM line-rate)
  - PE transposes 128x128 blocks into channel-major PSUM tiles
    pin[128c, 3+512 l]; the 3-column halo comes from a tiny PE re-transpose
    of the previous chunk's last 3 rows (zeros via memset at the edges)
  - conv taps are shifted free-dim slices of pin:
      acc  = Copy(pin[:, 0:512]*w0 + bias)      (ACT, per-partition scale+bias)
      acc += pin[:, k:512+k]*wk                 (DVE scalar_tensor_tensor), or
                                                (ACT mult + GPSIMD add) for a
                                                subset of c-blocks to balance
  - PE transposes acc back -> PSUM -> ACT copies -> contiguous DMA out
"""

import sys

for _p in ("/opt/trn_rl_repo",):
    if _p not in sys.path:
        sys.path.insert(0, _p)

import numpy as np

import concourse.bass as bass  # noqa: F401  (registers rust bindings)
import concourse.tile as tile
from concourse import bacc, mybir
from concourse.bass_utils import run_bass_kernel_spmd

F32 = mybir.dt.float32
MULT = mybir.AluOpType.mult
ADD = mybir.AluOpType.add
COPY = mybir.ActivationFunctionType.Copy

B, L, C, K, PAD = 8, 4096, 2048, 4, 3
LOUT = L + 2 * PAD - K + 1  # 4099
NCB = C // 128  # 16 channel blocks
CHUNK = 512  # output rows per main chunk
NCHUNK = L // CHUNK  # 8; last chunk extended to cover the LOUT tail
TAIL = LOUT - L  # 3 extra output rows in the last chunk

# tap-1 routing: c-blocks with index < ACT_TAP1_CBS use ACT-mult + GPSIMD-add;
# the rest use a fused DVE scalar_tensor_tensor. Taps 2 and 3 are always DVE.
ACT_TAP1_CBS = 12


def _build_nc():
    nc = bacc.Bacc("TRN2", target_bir_lowering=False, num_devices=B)

    x_d = nc.dram_tensor("x", [L, C], F32, kind="ExternalInput")
    wt_d = nc.dram_tensor("wt", [128, NCB * K], F32, kind="ExternalInput")
    bt_d = nc.dram_tensor("bt", [128, NCB], F32, kind="ExternalInput")
    id_d = nc.dram_tensor("ident", [128, 128], F32, kind="ExternalInput")
    out_d = nc.dram_tensor("out", [LOUT, C], F32, kind="ExternalOutput")

    with tile.TileContext(nc) as tc:
        with (
            tc.tile_pool(name="const", bufs=1) as cpool,
            tc.tile_pool(name="xin", bufs=4) as xin_pool,
            tc.tile_pool(name="t1", bufs=4) as t1_pool,
            tc.tile_pool(name="acc", bufs=1) as acc_pool,
            tc.tile_pool(name="osb", bufs=2) as out_pool,
            tc.tile_pool(name="pin", bufs=2, space="PSUM") as pin_pool,
            tc.tile_pool(name="pout", bufs=2, space="PSUM") as pout_pool,
        ):
            wt_sb = cpool.tile([128, NCB * K], F32)
            nc.sync.dma_start(out=wt_sb[:], in_=wt_d[:])
            bt_sb = cpool.tile([128, NCB], F32)
            nc.sync.dma_start(out=bt_sb[:], in_=bt_d[:])
            ident = cpool.tile([128, 128], F32)
            nc.sync.dma_start(out=ident[:], in_=id_d[:])

            for ci in range(NCHUNK):
                last = ci == NCHUNK - 1
                r0 = ci * CHUNK
                width = CHUNK + (TAIL if last else 0)  # output rows this chunk
                pinw = width + PAD  # pin columns

                # ---- load 512 input rows as 2x 2MiB contiguous DMAs ----
                xs = []
                for pair in range(2):
                    t = xin_pool.tile([128, 2, C], F32, tag="xin")
                    src = x_d[r0 + pair * 256 : r0 + pair * 256 + 256, :]
                    nc.sync.dma_start(
                        out=t[:], in_=src.rearrange("(r p) c -> p r c", p=128)
                    )
                    xs.append(t)
                # 3 halo rows [r0-3, r0) re-read as a tiny DMA (24KB)
                halo_sb = None
                if ci > 0:
                    halo_sb = xin_pool.tile([PAD, C], F32, tag="halo")
                    nc.sync.dma_start(out=halo_sb[:], in_=x_d[r0 - PAD : r0, :])

                # ---- per c-block: transpose into PSUM, then conv taps ----
                # PE writes must not cross the 2KB PSUM bank boundary, so the
                # data starts at column BASE=125: the 3 halo columns occupy
                # [125,128) and each 128-wide transpose lands bank-aligned.
                BASE = 128 - PAD
                accs = []
                for cb in range(NCB):
                    cs = slice(cb * 128, (cb + 1) * 128)
                    pin = pin_pool.tile([128, BASE + pinw], F32, tag="pin")
                    # halo: x rows [r0-3, r0) transposed into pin[:, BASE:BASE+3]
                    if ci == 0:
                        nc.vector.memset(pin[:, BASE : BASE + PAD], 0.0)
                    else:
                        nc.tensor.transpose(
                            pin[:, BASE : BASE + PAD],
                            halo_sb[0:PAD, cs],
                            ident[0:PAD, 0:PAD],
                        )
                    for lb in range(4):
                        nc.tensor.transpose(
                            pin[:, 128 + lb * 128 : 128 + (lb + 1) * 128],
                            xs[lb // 2][:, lb % 2, cs],
                            ident[:],
                        )
                    if last:
                        # virtual zero-pad rows beyond the end of x
                        nc.vector.memset(pin[:, 128 + CHUNK : BASE + pinw], 0.0)

                    wk = lambda k: wt_sb[:, cb * K + k : cb * K + k + 1]

                    acc = acc_pool.tile([128, width], F32, tag=f"acc{cb}")
                    # tap 0 on ACT: acc = pin[:, 0:w]*w0 + bias
                    # (Identity, not Copy: Copy rejects an AP bias)
                    nc.scalar.activation(
                        out=acc[:],
                        in_=pin[:, BASE : BASE + width],
                        func=mybir.ActivationFunctionType.Identity,
                        scale=wk(0),
                        bias=bt_sb[:, cb : cb + 1],
                    )
                    # taps 2, 3 fused on DVE
                    for k in (2, 3):
                        nc.vector.scalar_tensor_tensor(
                            out=acc[:],
                            in0=pin[:, BASE + k : BASE + k + width],
                            scalar=wk(k),
                            in1=acc[:],
                            op0=MULT,
                            op1=ADD,
                        )
                    # tap 1: ACT-mult + GPSIMD-add for most c-blocks
                    if cb < ACT_TAP1_CBS:
                        t1 = t1_pool.tile([128, width], F32, tag="t1")
                        nc.scalar.activation(
                            out=t1[:],
                            in_=pin[:, BASE + 1 : BASE + 1 + width],
                            func=COPY,
                            scale=wk(1),
                        )
                        nc.gpsimd.tensor_add(out=acc[:], in0=acc[:], in1=t1[:])
                    else:
                        nc.vector.scalar_tensor_tensor(
                            out=acc[:],
                            in0=pin[:, BASE + 1 : BASE + 1 + width],
                            scalar=wk(1),
                            in1=acc[:],
                            op0=MULT,
                            op1=ADD,
                        )
                    accs.append(acc)

                prev_last_x = xs[1]

                # ---- transpose back + store ----
                for pair in range(2):
                    osb = out_pool.tile([128, 2, C], F32, tag="osb")
                    for sub in range(2):
                        lb = pair * 2 + sub
                        for ch in range(2):
                            po = pout_pool.tile([128, 1024], F32, tag="pout")
                            for j in range(8):
                                cb = ch * 8 + j
                                nc.tensor.transpose(
                                    po[:, j * 128 : (j + 1) * 128],
                                    accs[cb][:, lb * 128 : (lb + 1) * 128],
                                    ident[:],
                                )
                            nc.scalar.copy(
                                out=osb[:, sub, ch * 1024 : (ch + 1) * 1024],
                                in_=po[:],
                            )
                    dst = out_d[r0 + pair * 256 : r0 + pair * 256 + 256, :]
                    nc.sync.dma_start(
                        out=dst.rearrange("(r p) c -> p r c", p=128), in_=osb[:]
                    )

                if last:
                    # final TAIL output rows [L, LOUT)
                    osb3 = out_pool.tile([TAIL, C], F32, tag="osb")
                    for ch in range(2):
                        po = pout_pool.tile([TAIL, 1024], F32, tag="pout")
                        for j in range(8):
                            cb = ch * 8 + j
                            nc.tensor.transpose(
                                po[:, j * 128 : (j + 1) * 128],
                                accs[cb][:, CHUNK : CHUNK + TAIL],
                                ident[:],
                            )
                        nc.scalar.copy(
                            out=osb3[:, ch * 1024 : (ch + 1) * 1024], in_=po[:]
                        )
                    nc.sync.dma_start(out=out_d[L:LOUT, :], in_=osb3[:])

    nc.compile()
    return nc


_NC_CACHE = None


def _get_nc():
    global _NC_CACHE
    if _NC_CACHE is None:
        _NC_CACHE = _build_nc()
    return _NC_CACHE


def _const_inputs(weight, bias):
    # wt[p, cb*K + k] = weight[cb*128 + p, k]
    wt = np.ascontiguousarray(
        weight.astype(np.float32).reshape(NCB, 128, K).transpose(1, 0, 2)
    ).reshape(128, NCB * K)
    bt = np.ascontiguousarray(bias.astype(np.float32).reshape(NCB, 128).T)
    ident = np.eye(128, dtype=np.float32)
    return wt, bt, ident


def _in_maps(x, weight, bias):
    wt, bt, ident = _const_inputs(weight, bias)
    return [
        {
            "x": np.ascontiguousarray(x[b], dtype=np.float32),
            "wt": wt,
            "bt": bt,
            "ident": ident,
        }
        for b in range(B)
    ]


def kernel(x, weight, bias):
    assert x.shape == (B, L, C) and weight.shape == (C, K) and bias.shape == (C,)
    nc = _get_nc()
    in_maps = _in_maps(x, weight, bias)
    res = run_bass_kernel_spmd(nc, in_maps, core_ids=list(range(B)))
    return np.stack([res.results[b]["out"] for b in range(B)], axis=0)


if __name__ == "__main__":
    rng = np.random.default_rng(0)
    x = rng.standard_normal((B, L, C), dtype=np.float32)
    w = (rng.standard_normal((C, K)) * 0.1).astype(np.float32)
    bias = (rng.standard_normal((C,)) * 0.1).astype(np.float32)
    out = kernel(x, w, bias)
    print("out", out.shape, out.dtype)
